# revision 7
# baseline (speedup 1.0000x reference)
"""Trainium2 Bass kernel for nn_CrossAttention_45724221833727.

Data-parallel over batch: 8 samples -> 8 NeuronCores, one [S=2048, D=512]
cross-attention problem per core. Weights/pos replicated.

Per-core pipeline (fp32 throughout):
  A) load evo, PE-transpose to evoT [d, s] (zero-padded for the conv)
  B) depthwise conv1d along s in [d, s] layout (ACT muls + DVE adds)
  C) QT[e, i] = WqT-matmuls(evoT) + pos (PE-transpose folded into the same
     PSUM accumulation) + bq (K=1 matmul fold)
  D) KVT[o, s] and KV[s, o] both as matmuls of kvdw (pw_b folded as K=1
     matmul); ke = KV + evo (residual, in-place into evo tiles)
  E) attention: scoresT[j, i] -> exp (fused scale, no max-subtract; inputs
     are sub-unit-variance so |scores|*scale < ~3) -> PV + row-sum l
     accumulated in PSUM over j-blocks -> out = out_unnorm/l + ke
"""

import math

import numpy as np

import concourse.bass as bass
import concourse.mybir as mybir
import concourse.tile as tile
from concourse.bass_utils import run_bass_kernel_spmd
from concourse.masks import make_identity

F32 = mybir.dt.float32
P = 128
S = 2048
D = 512
KS = 6
N_CORES = 8
SB = S // P      # 16 s-blocks
DC = D // P      # 4 d-chunks
IT = S // 512    # 4 i-tiles of 512
ACT_EXP = mybir.ActivationFunctionType.Exp
ACT_COPY = mybir.ActivationFunctionType.Copy

_COMPILED = {}


def _install_tail_drain_patch():
    """This container's walrus build only accepts ONE sync wait per
    instruction; TileContext's tail drain carries one wait per live
    engine/DMA-queue. Split them across single-wait NOPs."""
    if getattr(tile.TileContext, "_tail_patch_installed", False):
        return

    def _patched_drain_and_barrier(self, tick_clock, wait_clock):
        from concourse.tile import ScopedClock

        drain_inst = self.nc.sync.drain()
        wait_clock.add_sem_waits(
            drain_inst.ins, ScopedClock({None: tick_clock.global_clock})
        )
        si = drain_inst.ins.sync_info
        waits = list(si.on_wait) if si and si.on_wait else []
        if len(waits) > 1:
            drain_inst.ins.sync_info = mybir.SyncInfo(
                on_wait=[], on_update=list(si.on_update or [])
            )
            for i, w in enumerate(waits):
                nop = self.nc.sync.nop(nofuse=True, hint=f"tail_wait_{i}")
                nop.ins.sync_info = mybir.SyncInfo(on_wait=[w], on_update=[])

        self.nc.all_engine_barrier()
        assert self.sems is not None
        popped = self.nc._tile_sem_poison_stack.pop()
        assert popped is self._sem_poison
        self.nc.clear_and_free_semaphores(list(self.sems.allocated().values()))
        self.nc.all_engine_barrier()

    tile.TileContext._drain_and_barrier = _patched_drain_and_barrier
    tile.TileContext._tail_patch_installed = True


def _split_multi_waits(nc):
    """Walrus in this container accepts at most ONE sync wait per
    instruction. Hoist extra waits onto single-wait NOPs inserted just
    before the instruction in the same engine's stream (equivalent
    semantics: the engine stalls at the NOP instead)."""
    ctr = [0]
    for fn in nc.m.functions:
        for blk in fn.blocks:
            insts = list(blk.instructions)
            out = []
            changed = False
            for inst in insts:
                si = inst.sync_info
                if si is not None and si.on_wait and len(si.on_wait) > 1:
                    waits = list(si.on_wait)
                    for w in waits[:-1]:
                        nop = mybir.InstNoOp(
                            name=f"splitw-{ctr[0]}", ins=[], outs=[]
                        )
                        ctr[0] += 1
                        nop.engine = inst.engine
                        nop.sync_info = mybir.SyncInfo(on_wait=[w], on_update=[])
                        out.append(nop)
                    inst.sync_info = mybir.SyncInfo(
                        on_wait=[waits[-1]], on_update=list(si.on_update or [])
                    )
                    changed = True
                out.append(inst)
            if changed:
                blk.instructions = out
    return nc


def _build():
    _install_tail_drain_patch()
    nc = bass.Bass()
    evo_d = nc.dram_tensor("evo", [S, D], F32, kind="ExternalInput")
    wqt_d = nc.dram_tensor("wqt", [D, D], F32, kind="ExternalInput")   # Wq.T [d, e]
    bq_d = nc.dram_tensor("bq", [D], F32, kind="ExternalInput")
    pos_d = nc.dram_tensor("pos", [S, D], F32, kind="ExternalInput")
    dww_d = nc.dram_tensor("dww", [D, KS], F32, kind="ExternalInput")  # depthwise taps
    pwt_d = nc.dram_tensor("pwt", [D, D], F32, kind="ExternalInput")   # pw_w.T [d, o]
    pwb_d = nc.dram_tensor("pwb", [D], F32, kind="ExternalInput")      # pw_b + pw_w@dw_b
    out_d = nc.dram_tensor("out", [S, D], F32, kind="ExternalOutput")

    scale = 1.0 / math.sqrt(float(D))
    PAD = 2056  # 2 left pad + 2048 + 3 right pad, rounded up

    with tile.TileContext(nc) as tc:
        # Long-lived pools on the LEFT side; phase-transient pools on the
        # RIGHT side, released LIFO so the stack allocator reclaims them.
        cpool = tc.alloc_tile_pool(name="consts", bufs=1, side="left")
        epool = tc.alloc_tile_pool(name="evo", bufs=1, side="left")

        # ---- constants ----
        ident = cpool.tile([P, P], F32, tag="ident")
        make_identity(nc, ident)
        ones_col = cpool.tile([P, 1], F32, tag="ones_col")
        nc.vector.memset(ones_col[:], 1.0)
        ones_row = cpool.tile([1, 512], F32, tag="ones_row")
        nc.vector.memset(ones_row[:], 1.0)
        bq_row = cpool.tile([1, D], F32, tag="bq_row")
        nc.sync.dma_start(out=bq_row[:], in_=bq_d[None, :])
        pwb_row = cpool.tile([1, D], F32, tag="pwb_row")
        nc.sync.dma_start(out=pwb_row[:], in_=pwb_d[None, :])
        dww_sb = cpool.tile([P, DC * KS], F32, tag="dww")
        for dc in range(DC):
            nc.sync.dma_start(
                out=dww_sb[:, dc * KS:(dc + 1) * KS],
                in_=dww_d[dc * P:(dc + 1) * P, :],
            )
        wqt_sb = [cpool.tile([P, D], F32, tag=f"wqt{dc}", name=f"wqt{dc}") for dc in range(DC)]
        pwt_sb = [cpool.tile([P, D], F32, tag=f"pwt{dc}", name=f"pwt{dc}") for dc in range(DC)]
        for dc in range(DC):
            nc.sync.dma_start(out=wqt_sb[dc][:], in_=wqt_d[dc * P:(dc + 1) * P, :])
            nc.sync.dma_start(out=pwt_sb[dc][:], in_=pwt_d[dc * P:(dc + 1) * P, :])

        # ---- evo tiles (later become ke = KV + evo) ----
        evo_sb = [epool.tile([P, D], F32, tag=f"evo{sb}", name=f"evo{sb}") for sb in range(SB)]
        for sb in range(SB):
            nc.sync.dma_start(out=evo_sb[sb][:], in_=evo_d[sb * P:(sb + 1) * P, :])

        # ---- phase A (evoT) + B (depthwise conv) ----
        dwpool = tc.alloc_tile_pool(name="kvdw", bufs=1, side="right")
        etpool = tc.alloc_tile_pool(name="evoT", bufs=1, side="right")
        evoT = [etpool.tile([P, PAD], F32, tag=f"evoT{dc}", name=f"evoT{dc}") for dc in range(DC)]
        kvdw = [dwpool.tile([P, S], F32, tag=f"kvdw{dc}", name=f"kvdw{dc}") for dc in range(DC)]

        psa = tc.alloc_tile_pool(name="tpsA", bufs=4, space="PSUM")
        dwtmp = tc.alloc_tile_pool(name="dwtmp", bufs=3, side="right")
        for dc in range(DC):
            nc.vector.memset(evoT[dc][:, 0:2], 0.0)
            nc.vector.memset(evoT[dc][:, 2 + S:PAD], 0.0)
            for sb in range(SB):
                ps = psa.tile([P, P], F32, tag="tp", name="tp")
                nc.tensor.transpose(
                    ps[:], evo_sb[sb][:, dc * P:(dc + 1) * P], ident[:]
                )
                nc.vector.tensor_copy(
                    evoT[dc][:, 2 + sb * P:2 + (sb + 1) * P], ps[:]
                )
            # depthwise conv along s for this d-chunk
            acc = kvdw[dc]
            t0 = dwtmp.tile([P, S], F32, tag="dwt", name="dwt")
            nc.scalar.activation(
                t0[:], evoT[dc][:, 0:S], ACT_COPY,
                scale=dww_sb[:, dc * KS:dc * KS + 1],
            )
            t1 = dwtmp.tile([P, S], F32, tag="dwt", name="dwt1")
            nc.scalar.activation(
                t1[:], evoT[dc][:, 1:1 + S], ACT_COPY,
                scale=dww_sb[:, dc * KS + 1:dc * KS + 2],
            )
            nc.vector.tensor_add(acc[:], t0[:], t1[:])
            for k in range(2, KS):
                tk = dwtmp.tile([P, S], F32, tag="dwt", name="dwtk")
                nc.scalar.activation(
                    tk[:], evoT[dc][:, k:k + S], ACT_COPY,
                    scale=dww_sb[:, dc * KS + k:dc * KS + k + 1],
                )
                nc.vector.tensor_add(acc[:], acc[:], tk[:])
        dwtmp.release()
        psa.release()

        # ---- phase C: QT = WqT.T @ evoT + posT + bq ----
        qpool = tc.alloc_tile_pool(name="qt", bufs=1, side="left")
        qt = [qpool.tile([P, S], F32, tag=f"qt{ec}", name=f"qt{ec}") for ec in range(DC)]
        psc = tc.alloc_tile_pool(name="tpsC", bufs=2, space="PSUM")
        pospool = tc.alloc_tile_pool(name="pos", bufs=8, side="right")
        for it in range(IT):
            pos_tiles = []
            for sub in range(4):
                pt_ = pospool.tile([P, D], F32, tag="pos", name="pos")
                sb = it * 4 + sub
                nc.sync.dma_start(out=pt_[:], in_=pos_d[sb * P:(sb + 1) * P, :])
                pos_tiles.append(pt_)
            for ec in range(DC):
                ps = psc.tile([P, 512], F32, tag="qps", name="qps")
                for dc in range(DC):
                    nc.tensor.matmul(
                        ps[:], wqt_sb[dc][:, ec * P:(ec + 1) * P],
                        evoT[dc][:, 2 + it * 512:2 + (it + 1) * 512],
                        start=(dc == 0), stop=False,
                    )
                nc.tensor.matmul(
                    ps[:], bq_row[0:1, ec * P:(ec + 1) * P],
                    ones_row[0:1, :512], start=False, stop=False,
                )
                for sub in range(4):
                    nc.tensor.matmul(
                        ps[:, sub * P:(sub + 1) * P],
                        pos_tiles[sub][:, ec * P:(ec + 1) * P],
                        ident[:], is_transpose=True,
                        start=False, stop=(sub == 3),
                    )
                nc.scalar.copy(qt[ec][:, it * 512:(it + 1) * 512], ps[:])
        pospool.release()
        etpool.release()
        psc.release()

        # ---- phase D: KVT, KV, ke ----
        kvtpool = tc.alloc_tile_pool(name="kvt", bufs=1, side="left")
        kvpool = tc.alloc_tile_pool(name="kv", bufs=1, side="left")
        kvt = [kvtpool.tile([P, S], F32, tag=f"kvt{ob}", name=f"kvt{ob}") for ob in range(DC)]
        kv = [kvpool.tile([P, D], F32, tag=f"kv{sb}", name=f"kv{sb}") for sb in range(SB)]
        psd = tc.alloc_tile_pool(name="tpsD", bufs=2, space="PSUM")
        for ob in range(DC):
            for st in range(IT):
                ps = psd.tile([P, 512], F32, tag="kvtps", name="kvtps")
                for dc in range(DC):
                    nc.tensor.matmul(
                        ps[:], pwt_sb[dc][:, ob * P:(ob + 1) * P],
                        kvdw[dc][:, st * 512:(st + 1) * 512],
                        start=(dc == 0), stop=False,
                    )
                nc.tensor.matmul(
                    ps[:], pwb_row[0:1, ob * P:(ob + 1) * P],
                    ones_row[0:1, :512], start=False, stop=True,
                )
                nc.scalar.copy(kvt[ob][:, st * 512:(st + 1) * 512], ps[:])
        for sb in range(SB):
            ps = psd.tile([P, 512], F32, tag="kvps", name="kvps")
            for dc in range(DC):
                nc.tensor.matmul(
                    ps[:], kvdw[dc][:, sb * P:(sb + 1) * P],
                    pwt_sb[dc][:], start=(dc == 0), stop=False,
                )
            nc.tensor.matmul(
                ps[:], ones_row[0:1, :P], pwb_row[0:1, :],
                start=False, stop=True,
            )
            nc.scalar.copy(kv[sb][:], ps[:])
            # ke = evo + kv, in place into evo tile
            nc.vector.tensor_add(evo_sb[sb][:], evo_sb[sb][:], kv[sb][:])
        kvdw_released = True
        dwpool.release()
        psd.release()

        # ---- phase E: attention ----
        pss = tc.alloc_tile_pool(name="spsE", bufs=2, space="PSUM")
        pso = tc.alloc_tile_pool(name="opsE", bufs=4, space="PSUM")
        psl = tc.alloc_tile_pool(name="lpsE", bufs=2, space="PSUM")
        ptpool = tc.alloc_tile_pool(name="ptE", bufs=3, side="right")
        epipool = tc.alloc_tile_pool(name="epi", bufs=4, side="right")
        for ig in range(IT):
            out_ps = [pso.tile([P, 512], F32, tag="ops", name="ops") for _ in range(4)]
            l_ps = psl.tile([P, 4], F32, tag="lps", name="lps")
            for jb in range(SB):
                s_ps = pss.tile([P, 512], F32, tag="sps", name="sps")
                for ec in range(DC):
                    nc.tensor.matmul(
                        s_ps[:], kvt[ec][:, jb * P:(jb + 1) * P],
                        qt[ec][:, ig * 512:(ig + 1) * 512],
                        start=(ec == 0), stop=(ec == DC - 1),
                    )
                p_t = ptpool.tile([P, 512], F32, tag="pt", name="pt")
                nc.scalar.activation(p_t[:], s_ps[:], ACT_EXP, scale=scale)
                for ib in range(4):
                    nc.tensor.matmul(
                        out_ps[ib][:], p_t[:, ib * P:(ib + 1) * P],
                        kv[jb][:], start=(jb == 0), stop=(jb == SB - 1),
                    )
                    nc.tensor.matmul(
                        l_ps[:, ib:ib + 1], p_t[:, ib * P:(ib + 1) * P],
                        ones_col[:], start=(jb == 0 and ib == 0),
                        stop=(jb == SB - 1 and ib == 3),
                    )
            for ib in range(4):
                sb = ig * 4 + ib
                rl = epipool.tile([P, 1], F32, tag="rl", name="rl")
                nc.vector.reciprocal(rl[:], l_ps[:, ib:ib + 1])
                o_sb = epipool.tile([P, 512], F32, tag="osb", name="osb")
                nc.scalar.activation(
                    o_sb[:], out_ps[ib][:], ACT_COPY, scale=rl[:]
                )
                nc.vector.tensor_add(o_sb[:], o_sb[:], evo_sb[sb][:])
                nc.sync.dma_start(out=out_d[sb * P:(sb + 1) * P, :], in_=o_sb[:])
        epipool.release()
        ptpool.release()
        psl.release()
        pso.release()
        pss.release()
        kvpool.release()
        kvtpool.release()
        qpool.release()
        epool.release()
        cpool.release()

    _split_multi_waits(nc)
    return nc


def kernel(evo_local, Wq, bq, dw_w, dw_b, pw_w, pw_b, pos):
    evo_local = np.asarray(evo_local, dtype=np.float32)
    Wq = np.asarray(Wq, dtype=np.float32)
    bq = np.asarray(bq, dtype=np.float32)
    dw_w = np.asarray(dw_w, dtype=np.float32)
    dw_b = np.asarray(dw_b, dtype=np.float32)
    pw_w = np.asarray(pw_w, dtype=np.float32)
    pw_b = np.asarray(pw_b, dtype=np.float32)
    pos = np.asarray(pos, dtype=np.float32)

    if "nc" not in _COMPILED:
        _COMPILED["nc"] = _build()
    nc = _COMPILED["nc"]

    wqt = np.ascontiguousarray(Wq.T)                      # [d, e]
    pwt = np.ascontiguousarray(pw_w.T)                    # [d, o]
    pwb_eff = (pw_b + pw_w @ dw_b).astype(np.float32)     # fold depthwise bias
    dww = np.ascontiguousarray(dw_w[:, 0, :])             # [D, K]
    pos0 = np.ascontiguousarray(pos[0])                   # [S, D]

    in_maps = []
    for c in range(N_CORES):
        in_maps.append({
            "evo": np.ascontiguousarray(evo_local[c]),
            "wqt": wqt,
            "bq": bq,
            "pos": pos0,
            "dww": dww,
            "pwt": pwt,
            "pwb": pwb_eff,
        })
    res = run_bass_kernel_spmd(nc, in_maps, core_ids=list(range(N_CORES)))
    out = np.stack([res.results[c]["out"] for c in range(N_CORES)], axis=0)
    return out.astype(np.float32)
